# revision 9
# baseline (speedup 1.0000x reference)
"""Trainium2 Bass kernel for nn_NeuralSheet (S=80, N=6400, 16 relaxation steps).

Strategy (8 NeuronCores, tensor-parallel on the 6400x6400 GEMV):
- Row-shard the lateral matrices 800 rows/core using a residue layout
  (core k owns global rows g with (g mod 128) in [16k, 16k+16)) so the
  per-step AllGather of r lands directly in the PE's [128 x 50] lhsT layout.
- Only two distinct (t, u) combos occur over the 16 steps, so the device
  prebuilds two combined matrices in SBUF (fp16):
      W_A = E - 1.6*U            (steps 1-4)
      W_B = E - 0.6*U - 0.4*lri  where lri = row-normalized C*(1 - U/umax)
  and each step is one resident GEMV + pointwise tanh/relu + AllGather.
- Host passes transposed shards (contraction index major) so the PE can
  contract along partitions; all FLOPs (masks, normalization, matrix
  builds, GEMVs, activations) run on device.
"""
import sys
import os
import numpy as np

for _p in ("/opt/trn_rl_repo", "/root/.axon_site/_ro/trn_rl_repo"):
    if os.path.isdir(_p) and _p not in sys.path:
        sys.path.insert(0, _p)

import concourse.bass as bass  # noqa: E402,F401
import concourse.mybir as mybir  # noqa: E402
import concourse.tile as tile  # noqa: E402
from concourse import bacc, bass_isa  # noqa: E402
from concourse.bass_utils import run_bass_kernel_spmd  # noqa: E402

F32 = mybir.dt.float32
F16 = mybir.dt.float16
ALU = mybir.AluOpType
ACTF = mybir.ActivationFunctionType

NCORES = 8
S = 80
N = S * S            # 6400
NPC = N // NCORES    # 800 outputs per core
CH = N // 128        # 50 contraction chunks
ITERS = 16
NINV = ITERS // 3    # 5: steps 0..4 use W_A, 5..15 use W_B
II = 32 * 32         # 1024 afferent input size

_NC_CACHE = None
RUN_KWARGS = {}     # test harness may set {"trace": True, "tmpdir": ...}
LAST_RESULT = None


def build():
    nc = bacc.Bacc("TRN2", target_bir_lowering=False, debug=False,
                   num_devices=NCORES)
    ET = nc.dram_tensor("ET", [N, NPC], F32, kind="ExternalInput").ap()
    UT = nc.dram_tensor("UT", [N, NPC], F32, kind="ExternalInput").ap()
    CT = nc.dram_tensor("CT", [N, NPC], F32, kind="ExternalInput").ap()
    AT = nc.dram_tensor("AT", [II, NPC], F32, kind="ExternalInput").ap()
    XV = nc.dram_tensor("XV", [1, II], F32, kind="ExternalInput").ap()
    TH = nc.dram_tensor("TH", [1, NPC], F32, kind="ExternalInput").ap()
    RO = nc.dram_tensor("RO", [1, NPC], F32, kind="ExternalOutput").ap()

    rg = [list(range(NCORES))]

    with tile.TileContext(nc) as tc:
        with (
            tc.tile_pool(name="res", bufs=1) as res,        # resident W chunks
            tc.tile_pool(name="stream", bufs=2) as stream,  # fp32 streams
            tc.tile_pool(name="s16", bufs=2) as s16,        # fp16 streams
            tc.tile_pool(name="small", bufs=1) as small,    # vectors
            tc.tile_pool(name="itp", bufs=3) as itp,        # per-iter tiles
            tc.tile_pool(name="zbp", bufs=2) as zbp,
            tc.tile_pool(name="ps", bufs=2, space="PSUM") as ps,
            tc.tile_pool(name="pss", bufs=1, space="PSUM") as pss,
            tc.tile_pool(name="dram", bufs=3, space="DRAM") as dram,
            tc.tile_pool(name="dram1", bufs=1, space="DRAM") as dram1,
        ):
            # ---------------- afferent + step 0 (no GEMV) ----------------
            x16 = small.tile([128, II // 128], F16, tag="x16")
            nc.gpsimd.dma_start(
                x16[:], XV.rearrange("o (c p) -> (o p) c", p=128))
            th32 = small.tile([1, NPC], F32, tag="throf")
            nc.sync.dma_start(th32[:], TH[:, :])

            psAffA = pss.tile([1, 512], F32, tag="pssA")
            psAffB = pss.tile([1, NPC - 512], F32, tag="pssB")
            for c8 in range(II // 128):
                a16 = s16.tile([128, NPC], F16, tag="c16")
                nc.gpsimd.dma_start(a16[:], AT[c8 * 128:(c8 + 1) * 128, :])
                st = (c8 == 0)
                sp = (c8 == II // 128 - 1)
                nc.tensor.matmul(psAffA[:], x16[:, c8:c8 + 1], a16[:, 0:512],
                                 start=st, stop=sp)
                nc.tensor.matmul(psAffB[:], x16[:, c8:c8 + 1], a16[:, 512:NPC],
                                 start=st, stop=sp)

            # bp = (aff - thr) / 2 ; z_k = 2*(lat_k + bp); r = relu(tanh(z))
            bp = small.tile([1, NPC], F32, tag="bp")
            nc.vector.tensor_tensor(bp[:, 0:512], psAffA[:], th32[:, 0:512],
                                    ALU.subtract)
            nc.vector.tensor_tensor(bp[:, 512:NPC], psAffB[:],
                                    th32[:, 512:NPC], ALU.subtract)
            nc.vector.tensor_scalar_mul(bp[:], bp[:], 0.5)

            def pointwise_and_ship(it, psA, psB):
                """z'=lat+bp; r=relu(tanh(2z')); ship via AG (or RO if last)."""
                if it == 0:
                    zb = bp
                else:
                    zb = zbp.tile([1, NPC], F32, tag="zb")
                    nc.vector.tensor_tensor(zb[:, 0:512], psA[:],
                                            bp[:, 0:512], ALU.add)
                    nc.vector.tensor_tensor(zb[:, 512:NPC], psB[:],
                                            bp[:, 512:NPC], ALU.add)
                if it == ITERS - 1:
                    rof = small.tile([1, NPC], F32, tag="throf")
                    nc.scalar.activation(rof[:], zb[:], ACTF.Tanh, scale=2.0)
                    nc.vector.tensor_scalar_max(rof[:], rof[:], 0.0)
                    nc.sync.dma_start(RO[:, :], rof[:])
                    return None
                rn = zbp.tile([1, NPC], F16, tag="rn")
                nc.scalar.activation(rn[:], zb[:], ACTF.Tanh, scale=2.0)
                nc.vector.tensor_scalar_max(rn[:], rn[:], 0.0)
                agin = dram.tile([1, NPC], F16, tag="agin")
                nc.sync.dma_start(agin[:], rn[:])
                agout = dram.tile([NCORES, NPC], F16, tag="agout")
                nc.gpsimd.collective_compute(
                    "AllGather", ALU.bypass, replica_groups=rg,
                    ins=[agin.opt()], outs=[agout.opt()])
                r16 = itp.tile([128, CH], F16, tag="r16")
                nc.sync.dma_start(
                    r16[:], agout.rearrange("k (p c) -> (k p) c", p=16))
                return r16

            r16 = pointwise_and_ship(0, None, None)

            # -------- phase A: stream E,U; build W_A; keep U16; row max ----
            wa = [res.tile([128, NPC], F16, tag=f"wa{c}", name=f"wa{c}")
                  for c in range(CH)]
            u16 = [res.tile([128, NPC], F16, tag=f"u16_{c}", name=f"u16_{c}")
                   for c in range(CH)]
            rm = small.tile([128, NPC], F16, tag="rm")
            nc.gpsimd.memset(rm[:], 0.0)

            for c in range(CH):
                rows = slice(128 * c, 128 * c + 128)
                et = stream.tile([128, NPC], F32, tag="et")
                nc.sync.dma_start(et[:], ET[rows, :])
                ut = stream.tile([128, NPC], F32, tag="ut")
                nc.sync.dma_start(ut[:], UT[rows, :])
                nc.scalar.copy(u16[c][:], ut[:])            # cast fp32->fp16
                nc.vector.tensor_tensor(rm[:], rm[:], u16[c][:], ALU.max)
                nc.scalar.mul(ut[:], ut[:], -1.6)           # in place
                nc.vector.tensor_tensor(wa[c][:], et[:], ut[:], ALU.add)

            # umax over partitions -> replicated [128, NPC]; repinv = 1/umax
            rmax32 = small.tile([128, NPC], F32, tag="rmax32")
            nc.gpsimd.partition_all_reduce(rmax32[:], rm[:], 128,
                                           bass_isa.ReduceOp.max)
            nc.vector.reciprocal(rmax32[:], rmax32[:])
            repinv = small.tile([128, NPC], F16, tag="repinv")
            nc.scalar.copy(repinv[:], rmax32[:])

            # ---------------- GEMV helper ----------------
            def gemv(W, r16_t):
                psA = ps.tile([1, 512], F32, tag="psA")
                psB = ps.tile([1, NPC - 512], F32, tag="psB")
                for c in range(CH):
                    st = (c == 0)
                    sp = (c == CH - 1)
                    nc.tensor.matmul(psA[:], r16_t[:, c:c + 1],
                                     W[c][:, 0:512], start=st, stop=sp)
                    nc.tensor.matmul(psB[:], r16_t[:, c:c + 1],
                                     W[c][:, 512:NPC], start=st, stop=sp)
                return psA, psB

            # ---------------- B1: lri numerator + row sums ----------------
            lu_dram = dram1.tile([N, NPC], F16)
            psSA = pss.tile([1, 512], F32, tag="pssA")
            psSB = pss.tile([1, NPC - 512], F32, tag="pssB")
            ones16 = small.tile([128, 1], F16, tag="ones16")
            nc.vector.memset(ones16[:], 1.0)

            def emit_b1(c):
                c16 = s16.tile([128, NPC], F16, tag="c16")
                nc.gpsimd.dma_start(c16[:], CT[128 * c:128 * (c + 1), :])
                m16 = s16.tile([128, NPC], F16, tag="m16")
                nc.vector.tensor_tensor(m16[:], u16[c][:], repinv[:], ALU.mult)
                nc.scalar.activation(m16[:], m16[:], ACTF.Copy,
                                     bias=1.0, scale=-1.0)
                lu = s16.tile([128, NPC], F16, tag="lu")
                nc.vector.tensor_tensor(lu[:], c16[:], m16[:], ALU.mult)
                st = (c == 0)
                sp = (c == CH - 1)
                nc.tensor.matmul(psSA[:], ones16[:], lu[:, 0:512],
                                 start=st, stop=sp)
                nc.tensor.matmul(psSB[:], ones16[:], lu[:, 512:NPC],
                                 start=st, stop=sp)
                nc.sync.dma_start(lu_dram[128 * c:128 * (c + 1), :], lu[:])

            # ---------------- steps 1..3 on W_A, B1 interleaved ------------
            for it in range(1, NINV - 1):
                psA, psB = gemv(wa, r16)
                r16 = pointwise_and_ship(it, psA, psB)
                for c in range((it - 1) * 17, min(CH, it * 17)):
                    emit_b1(c)

            # rep_s = 0.4 / (s + 1e-11), replicated to [128, NPC] fp16
            sinv = small.tile([1, NPC], F32, tag="sinv")
            nc.vector.tensor_scalar(sinv[:, 0:512], psSA[:], 1e-11, None,
                                    ALU.add)
            nc.vector.tensor_scalar(sinv[:, 512:NPC], psSB[:], 1e-11, None,
                                    ALU.add)
            nc.vector.reciprocal(sinv[:], sinv[:])
            nc.vector.tensor_scalar_mul(sinv[:], sinv[:], 0.4)
            sinv16 = small.tile([1, NPC], F16, tag="sinv16")
            nc.vector.tensor_copy(sinv16[:], sinv[:])
            reps = small.tile([128, NPC], F16, tag="reps")
            nc.gpsimd.partition_broadcast(reps[:], sinv16[:], 128)

            # ------------ step 4 GEMV + B2 (wa -> W_B in place) ------------
            it = NINV - 1
            psA = ps.tile([1, 512], F32, tag="psA")
            psB = ps.tile([1, NPC - 512], F32, tag="psB")
            for c in range(CH):
                st = (c == 0)
                sp = (c == CH - 1)
                nc.tensor.matmul(psA[:], r16[:, c:c + 1], wa[c][:, 0:512],
                                 start=st, stop=sp)
                nc.tensor.matmul(psB[:], r16[:, c:c + 1], wa[c][:, 512:NPC],
                                 start=st, stop=sp)
                # B2 for chunk c right after its last W_A read
                lub = s16.tile([128, NPC], F16, tag="c16")
                nc.sync.dma_start(lub[:], lu_dram[128 * c:128 * (c + 1), :])
                eng = nc.vector
                eng.tensor_tensor(lub[:], lub[:], reps[:], ALU.mult)
                eng.tensor_tensor(wa[c][:], wa[c][:], u16[c][:], ALU.add)
                eng.tensor_tensor(wa[c][:], wa[c][:], lub[:], ALU.subtract)
            r16 = pointwise_and_ship(it, psA, psB)

            # ---------------- steps 5..15 on W_B ----------------
            for it in range(NINV, ITERS):
                psA, psB = gemv(wa, r16)
                r16 = pointwise_and_ship(it, psA, psB)

    nc.compile()
    return nc


def _get_nc():
    global _NC_CACHE
    if _NC_CACHE is None:
        _NC_CACHE = build()
    return _NC_CACHE


def kernel(input_crop, afferent_weights, lateral_weights_exc,
           untuned_inh, lateral_correlations, thresholds):
    nc = _get_nc()

    E = np.asarray(lateral_weights_exc, dtype=np.float32).reshape(N, N)
    U = np.asarray(untuned_inh, dtype=np.float32).reshape(N, N)
    C = np.asarray(lateral_correlations, dtype=np.float32).reshape(N, N)
    A = np.asarray(afferent_weights, dtype=np.float32).reshape(N, II)
    x = np.ascontiguousarray(
        np.asarray(input_crop, dtype=np.float32).reshape(1, II))
    th = np.asarray(thresholds, dtype=np.float32).reshape(N)

    in_maps = []
    rows_all = []
    for k in range(NCORES):
        # f-order: f = 50*p + c  <->  global row g = 128*c + 16*k + p
        p_idx = np.repeat(np.arange(16), CH)
        c_idx = np.tile(np.arange(CH), 16)
        rows = 128 * c_idx + 16 * k + p_idx
        rows_all.append(rows)
        in_maps.append({
            "ET": np.ascontiguousarray(E[rows, :].T),
            "UT": np.ascontiguousarray(U[rows, :].T),
            "CT": np.ascontiguousarray(C[rows, :].T),
            "AT": np.ascontiguousarray(A[rows, :].T),
            "XV": x,
            "TH": np.ascontiguousarray(th[rows][None, :]),
        })

    global LAST_RESULT
    kw = dict(RUN_KWARGS)
    res = run_bass_kernel_spmd(nc, in_maps, core_ids=list(range(NCORES)), **kw)
    LAST_RESULT = res

    r = np.zeros(N, dtype=np.float32)
    for k in range(NCORES):
        out = res.results[k]["RO"].reshape(NPC)
        r[rows_all[k]] = out
    return r.reshape(1, 1, S, S)


if __name__ == "__main__":
    rng = np.random.default_rng(0)
    out = kernel(
        input_crop=rng.uniform(size=(1, 1, 32, 32)).astype(np.float32),
        afferent_weights=rng.uniform(size=(N, 1, 32, 32)).astype(np.float32),
        lateral_weights_exc=rng.uniform(size=(N, 1, S, S)).astype(np.float32),
        untuned_inh=rng.uniform(size=(N, 1, S, S)).astype(np.float32),
        lateral_correlations=rng.uniform(size=(N, 1, S, S)).astype(np.float32),
        thresholds=np.zeros((1, 1, S, S), dtype=np.float32),
    )
    print(out.shape, out.dtype, float(out.mean()))


# revision 10
# speedup vs baseline: 1.0520x; 1.0520x over previous
"""Trainium2 Bass kernel for nn_NeuralSheet (S=80, N=6400, 16 relaxation steps).

Strategy (8 NeuronCores, tensor-parallel on the 6400x6400 GEMV):
- Row-shard the lateral matrices 800 rows/core using a residue layout
  (core k owns global rows g with (g mod 128) in [16k, 16k+16)) so the
  per-step AllGather of r lands directly in the PE's [128 x 50] lhsT layout.
- Only two distinct (t, u) combos occur over the 16 steps, so the device
  prebuilds two combined matrices in SBUF (fp16):
      W_A = E - 1.6*U            (steps 1-4)
      W_B = E - 0.6*U - 0.4*lri  where lri = row-normalized C*(1 - U/umax)
  and each step is one resident GEMV + pointwise tanh/relu + AllGather.
- GEMV uses grouped stationaries (3 r-chunks per LDWEIGHTS at psum bases
  0/32/64) so matmuls pipeline back to back; diagonal rows are summed by a
  selector matmul.
- Host passes transposed shards (contraction index major); all FLOPs run
  on device.
"""
import sys
import os
import numpy as np

for _p in ("/opt/trn_rl_repo", "/root/.axon_site/_ro/trn_rl_repo"):
    if os.path.isdir(_p) and _p not in sys.path:
        sys.path.insert(0, _p)

import concourse.bass as bass  # noqa: E402,F401
import concourse.mybir as mybir  # noqa: E402
import concourse.tile as tile  # noqa: E402
from concourse import bacc, bass_isa  # noqa: E402
from concourse.bass_utils import run_bass_kernel_spmd  # noqa: E402

F32 = mybir.dt.float32
F16 = mybir.dt.float16
ALU = mybir.AluOpType
ACTF = mybir.ActivationFunctionType

NCORES = 8
S = 80
N = S * S            # 6400
NPC = N // NCORES    # 800 outputs per core
CH = N // 128        # 50 contraction chunks
ITERS = 16
NINV = ITERS // 3    # 5: steps 0..4 use W_A, 5..15 use W_B
II = 32 * 32         # 1024 afferent input size
G = 3                # r-chunks per stationary group
BASES = [0, 32, 64]
NJUNK = 32           # HAM-warming junk matmuls per iteration gap

_NC_CACHE = None
RUN_KWARGS = {}     # test harness may set {"trace": True, "tmpdir": ...}
LAST_RESULT = None


def build():
    nc = bacc.Bacc("TRN2", target_bir_lowering=False, debug=False,
                   num_devices=NCORES)
    ET = nc.dram_tensor("ET", [N, NPC], F32, kind="ExternalInput").ap()
    UT = nc.dram_tensor("UT", [N, NPC], F32, kind="ExternalInput").ap()
    CT = nc.dram_tensor("CT", [N, NPC], F32, kind="ExternalInput").ap()
    AT = nc.dram_tensor("AT", [II, NPC], F32, kind="ExternalInput").ap()
    XV = nc.dram_tensor("XV", [1, II], F32, kind="ExternalInput").ap()
    TH = nc.dram_tensor("TH", [1, NPC], F32, kind="ExternalInput").ap()
    SEL = nc.dram_tensor("SEL", [67, 1], F32, kind="ExternalInput").ap()
    RO = nc.dram_tensor("RO", [1, NPC], F32, kind="ExternalOutput").ap()

    rg = [list(range(NCORES))]

    with tile.TileContext(nc) as tc:
        with (
            tc.tile_pool(name="res", bufs=1) as res,        # resident W chunks
            tc.tile_pool(name="stream", bufs=2) as stream,  # fp32 streams
            tc.tile_pool(name="s16", bufs=2) as s16,        # fp16 streams
            tc.tile_pool(name="small", bufs=1) as small,    # vectors
            tc.tile_pool(name="itp", bufs=3) as itp,        # per-iter tiles
            tc.tile_pool(name="zbp", bufs=2) as zbp,
            tc.tile_pool(name="ps", bufs=2, space="PSUM") as ps,
            tc.tile_pool(name="pss", bufs=1, space="PSUM") as pss,
            tc.tile_pool(name="dram", bufs=3, space="DRAM") as dram,
            tc.tile_pool(name="dram1", bufs=1, space="DRAM") as dram1,
        ):
            # ---------------- constants / afferent / step 0 ----------------
            x16 = small.tile([128, II // 128], F16, tag="x16")
            nc.gpsimd.dma_start(
                x16[:], XV.rearrange("o (c p) -> (o p) c", p=128))
            th32 = small.tile([1, NPC], F32, tag="throf")
            nc.sync.dma_start(th32[:], TH[:, :])
            sel16 = small.tile([67, 1], F16, tag="sel16")
            nc.gpsimd.dma_start(sel16[:], SEL[:, :])
            junkw = small.tile([128, 512], F16, tag="junkw")
            nc.vector.memset(junkw[:], 0.0)

            psAffA = pss.tile([1, 512], F32, tag="pssA")
            psAffB = pss.tile([1, NPC - 512], F32, tag="pssB")
            for c8 in range(II // 128):
                a16 = s16.tile([128, NPC], F16, tag="c16")
                nc.gpsimd.dma_start(a16[:], AT[c8 * 128:(c8 + 1) * 128, :])
                st = (c8 == 0)
                sp = (c8 == II // 128 - 1)
                nc.tensor.matmul(psAffA[:], x16[:, c8:c8 + 1], a16[:, 0:512],
                                 start=st, stop=sp)
                nc.tensor.matmul(psAffB[:], x16[:, c8:c8 + 1], a16[:, 512:NPC],
                                 start=st, stop=sp)

            # bp = (aff - thr)/2 ; z_k = 2*(lat_k + bp); r = relu(tanh(z))
            bp = small.tile([1, NPC], F32, tag="bp")
            nc.vector.tensor_tensor(bp[:, 0:512], psAffA[:], th32[:, 0:512],
                                    ALU.subtract)
            nc.vector.tensor_tensor(bp[:, 512:NPC], psAffB[:],
                                    th32[:, 512:NPC], ALU.subtract)
            nc.vector.tensor_scalar_mul(bp[:], bp[:], 0.5)

            def pointwise_and_ship(it, psA, psB):
                """z'=lat+bp; r=relu(tanh(2z')); ship via AG (or RO if last).
                psA/psB hold the GEMV result in row 0 (after selector MM)."""
                if it == 0:
                    zb = bp
                else:
                    zb = zbp.tile([1, NPC], F32, tag="zb")
                    nc.vector.tensor_tensor(zb[:, 0:512], psA[0:1, :],
                                            bp[:, 0:512], ALU.add)
                    nc.vector.tensor_tensor(zb[:, 512:NPC], psB[0:1, :],
                                            bp[:, 512:NPC], ALU.add)
                if it == ITERS - 1:
                    rof = small.tile([1, NPC], F32, tag="throf")
                    nc.scalar.activation(rof[:], zb[:], ACTF.Tanh, scale=2.0)
                    nc.vector.tensor_scalar_max(rof[:], rof[:], 0.0)
                    nc.sync.dma_start(RO[:, :], rof[:])
                    return None
                rn = zbp.tile([1, NPC], F16, tag="rn")
                nc.scalar.activation(rn[:], zb[:], ACTF.Tanh, scale=2.0)
                nc.vector.tensor_scalar_max(rn[:], rn[:], 0.0)
                agin = dram.tile([1, NPC], F16, tag="agin")
                nc.sync.dma_start(agin[:], rn[:])
                agout = dram.tile([NCORES, NPC], F16, tag="agout")
                nc.gpsimd.collective_compute(
                    "AllGather", ALU.bypass, replica_groups=rg,
                    ins=[agin.opt()], outs=[agout.opt()])
                r16 = itp.tile([128, CH], F16, tag="r16")
                nc.sync.dma_start(
                    r16[:], agout.rearrange("k (p c) -> (k p) c", p=16))
                return r16

            def emit_junk(n):
                pj = pss.tile([1, 512], F32, tag="junkp")
                for _ in range(n):
                    nc.tensor.matmul(pj[:], junkw[:, 0:1], junkw[:],
                                     start=True, stop=True,
                                     skip_group_check=True)

            r16 = pointwise_and_ship(0, None, None)

            # -------- phase A: stream E,U; build W_A; keep U16; row max ----
            wa = [res.tile([128, NPC], F16, tag=f"wa{c}", name=f"wa{c}")
                  for c in range(CH)]
            u16 = [res.tile([128, NPC], F16, tag=f"u16_{c}", name=f"u16_{c}")
                   for c in range(CH)]
            rm = small.tile([128, NPC], F16, tag="rm")
            nc.gpsimd.memset(rm[:], 0.0)

            for c in range(CH):
                rows = slice(128 * c, 128 * c + 128)
                et = stream.tile([128, NPC], F32, tag="et")
                nc.sync.dma_start(et[:], ET[rows, :])
                nc.gpsimd.dma_start(u16[c][:], UT[rows, :])  # cast f32->f16
                t2 = stream.tile([128, NPC], F32, tag="t2")
                nc.scalar.mul(t2[:], u16[c][:], -1.6)
                nc.vector.tensor_tensor(wa[c][:], et[:], t2[:], ALU.add)
                nc.vector.tensor_tensor(rm[:], rm[:], u16[c][:], ALU.max)

            # umax over partitions (replicated); repinv = 1/umax  (fp16)
            rmax32 = stream.tile([128, NPC], F32, tag="t2")
            nc.gpsimd.partition_all_reduce(rmax32[:], rm[:], 128,
                                           bass_isa.ReduceOp.max)
            nc.vector.reciprocal(rmax32[:], rmax32[:])
            repinv = small.tile([128, NPC], F16, tag="repinv")
            nc.scalar.copy(repinv[:], rmax32[:])

            # ---------------- grouped GEMV ----------------
            def gemv_mms(W, r16_t, psA, psB):
                ng = (CH + G - 1) // G
                for g in range(ng):
                    m = min(G, CH - g * G)
                    # group 0 uses a wide stationary so psum rows [0:96)
                    # are all initialized (finite) for the [0:67] extraction
                    mm = 32 if g == 0 else m
                    lhsT = r16_t[:, g * G:g * G + mm]
                    for j in range(m):
                        c = g * G + j
                        st = (g == 0)
                        sp = (c + G >= CH)
                        b = BASES[j]
                        nc.tensor.matmul(psA[b:b + mm, :], lhsT,
                                         W[c][:, 0:512], start=st, stop=sp,
                                         skip_group_check=True)
                        nc.tensor.matmul(psB[b:b + mm, :], lhsT,
                                         W[c][:, 512:NPC], start=st, stop=sp,
                                         skip_group_check=True)
                    yield g

            def gemv_extract(psA, psB):
                # sum rows {0,33,66} via selector matmul into row 0
                exA = zbp.tile([67, 512], F16, tag="exA")
                nc.scalar.copy(exA[:], psA[0:67, :])
                exB = zbp.tile([67, NPC - 512], F16, tag="exB")
                nc.vector.tensor_copy(exB[:], psB[0:67, :])
                nc.tensor.matmul(psA[0:1, :], sel16[:], exA[:],
                                 start=True, stop=True, skip_group_check=True)
                nc.tensor.matmul(psB[0:1, :], sel16[:], exB[:],
                                 start=True, stop=True, skip_group_check=True)

            def gemv(W, r16_t):
                psA = ps.tile([128, 512], F32, tag="psA")
                psB = ps.tile([128, NPC - 512], F32, tag="psB")
                for _ in gemv_mms(W, r16_t, psA, psB):
                    pass
                gemv_extract(psA, psB)
                return psA, psB

            # ---------------- B1: lri numerator, row sums, q16 -------------
            lu_dram = dram1.tile([N, NPC], F16)
            s_dram = dram1.tile([128, NPC], F16)

            def emit_b1(c):
                c16 = s16.tile([128, NPC], F16, tag="c16")
                nc.gpsimd.dma_start(c16[:], CT[128 * c:128 * (c + 1), :])
                m16 = s16.tile([128, NPC], F16, tag="m16")
                nc.vector.tensor_tensor(m16[:], u16[c][:], repinv[:], ALU.mult)
                nc.scalar.activation(m16[:], m16[:], ACTF.Copy,
                                     bias=1.0, scale=-1.0)
                lu = s16.tile([128, NPC], F16, tag="lu")
                nc.vector.tensor_tensor(lu[:], c16[:], m16[:], ALU.mult)
                nc.sync.dma_start(lu_dram[128 * c:128 * (c + 1), :], lu[:])
                if c == 0:
                    nc.gpsimd.dma_start(s_dram[:], lu[:])
                else:
                    nc.gpsimd.dma_start(s_dram[:], lu[:], accum_op=ALU.add)
                # q16: u16[c] <- wa[c] + u16[c]   (used by B2)
                nc.vector.tensor_tensor(u16[c][:], wa[c][:], u16[c][:],
                                        ALU.add)

            # ---------------- steps 1..3 on W_A, B1 interleaved ------------
            for it in range(1, NINV - 1):
                psA, psB = gemv(wa, r16)
                r16 = pointwise_and_ship(it, psA, psB)
                emit_junk(NJUNK)
                for c in range((it - 1) * 17, min(CH, it * 17)):
                    emit_b1(c)

            # rep_s = 0.4 / (s + 1e-11) replicated [128, NPC] fp16
            s16b = small.tile([128, NPC], F16, tag="rm")
            nc.sync.dma_start(s16b[:], s_dram[:])
            srep = stream.tile([128, NPC], F32, tag="t2")
            nc.gpsimd.partition_all_reduce(srep[:], s16b[:], 128,
                                           bass_isa.ReduceOp.add)
            nc.vector.tensor_scalar(srep[:], srep[:], 1e-11, None, ALU.add)
            nc.vector.reciprocal(srep[:], srep[:])
            reps = small.tile([128, NPC], F16, tag="reps")
            nc.scalar.mul(reps[:], srep[:], 0.4)

            # ------------ step 4 GEMV + B2 (wa -> W_B in place) ------------
            it = NINV - 1
            psA = ps.tile([128, 512], F32, tag="psA")
            psB = ps.tile([128, NPC - 512], F32, tag="psB")
            for g in gemv_mms(wa, r16, psA, psB):
                m = min(G, CH - g * G)
                for j in range(m):
                    # B2 for chunk c after its last W_A read:
                    # wa[c] = q16[c] - rep_s * lu[c]
                    c = g * G + j
                    lub = s16.tile([128, NPC], F16, tag="c16")
                    nc.sync.dma_start(lub[:],
                                      lu_dram[128 * c:128 * (c + 1), :])
                    nc.vector.tensor_tensor(lub[:], lub[:], reps[:], ALU.mult)
                    nc.vector.tensor_tensor(wa[c][:], u16[c][:], lub[:],
                                            ALU.subtract)
            gemv_extract(psA, psB)
            r16 = pointwise_and_ship(it, psA, psB)
            emit_junk(NJUNK)

            # ---------------- steps 5..15 on W_B ----------------
            for it in range(NINV, ITERS):
                psA, psB = gemv(wa, r16)
                r16 = pointwise_and_ship(it, psA, psB)
                if it != ITERS - 1:
                    emit_junk(NJUNK)

    nc.compile()
    return nc


def _get_nc():
    global _NC_CACHE
    if _NC_CACHE is None:
        _NC_CACHE = build()
    return _NC_CACHE


def kernel(input_crop, afferent_weights, lateral_weights_exc,
           untuned_inh, lateral_correlations, thresholds):
    nc = _get_nc()

    E = np.asarray(lateral_weights_exc, dtype=np.float32).reshape(N, N)
    U = np.asarray(untuned_inh, dtype=np.float32).reshape(N, N)
    C = np.asarray(lateral_correlations, dtype=np.float32).reshape(N, N)
    A = np.asarray(afferent_weights, dtype=np.float32).reshape(N, II)
    x = np.ascontiguousarray(
        np.asarray(input_crop, dtype=np.float32).reshape(1, II))
    th = np.asarray(thresholds, dtype=np.float32).reshape(N)
    sel = np.zeros((67, 1), np.float32)
    sel[[0, 33, 66], 0] = 1.0

    in_maps = []
    rows_all = []
    for k in range(NCORES):
        # f-order: f = 50*p + c  <->  global row g = 128*c + 16*k + p
        p_idx = np.repeat(np.arange(16), CH)
        c_idx = np.tile(np.arange(CH), 16)
        rows = 128 * c_idx + 16 * k + p_idx
        rows_all.append(rows)
        in_maps.append({
            "ET": np.ascontiguousarray(E[rows, :].T),
            "UT": np.ascontiguousarray(U[rows, :].T),
            "CT": np.ascontiguousarray(C[rows, :].T),
            "AT": np.ascontiguousarray(A[rows, :].T),
            "XV": x,
            "TH": np.ascontiguousarray(th[rows][None, :]),
            "SEL": sel,
        })

    global LAST_RESULT
    kw = dict(RUN_KWARGS)
    res = run_bass_kernel_spmd(nc, in_maps, core_ids=list(range(NCORES)), **kw)
    LAST_RESULT = res

    r = np.zeros(N, dtype=np.float32)
    for k in range(NCORES):
        out = res.results[k]["RO"].reshape(NPC)
        r[rows_all[k]] = out
    return r.reshape(1, 1, S, S)


if __name__ == "__main__":
    rng = np.random.default_rng(0)
    out = kernel(
        input_crop=rng.uniform(size=(1, 1, 32, 32)).astype(np.float32),
        afferent_weights=rng.uniform(size=(N, 1, 32, 32)).astype(np.float32),
        lateral_weights_exc=rng.uniform(size=(N, 1, S, S)).astype(np.float32),
        untuned_inh=rng.uniform(size=(N, 1, S, S)).astype(np.float32),
        lateral_correlations=rng.uniform(size=(N, 1, S, S)).astype(np.float32),
        thresholds=np.zeros((1, 1, S, S), dtype=np.float32),
    )
    print(out.shape, out.dtype, float(out.mean()))
